# revision 10
# baseline (speedup 1.0000x reference)
"""Trainium2 Bass kernel for KerasCrossAttentionModule (B=8, S=4096, D=256).

Sharding: data-parallel over batch B across 8 NeuronCores (1 batch/core).

Host prep (layout only, same class as the host transposes the problem needs
anyway): pos embeddings pre-added, operands cast to fp16, V pre-tiled so the
device sees three dense fp16 streams.

Per-core device math:
    scoresT[k, i] = sum_d kh[d, k] * qh[d, i]       (PE, fp16 in / fp32 acc)
    E = exp(scale * scoresT)                        (ACT, fp32 -> fp16)
    denom[i] = sum_k E[k, i]                        (DVE adds + GpSimd reduce)
    outT[d, i] = sum_k v[k, d] * E[k, i] / denom[i] (PE + DVE)

Pipelining: scores are emitted 2 key-chunks ahead of the AV matmuls so the
exp() latency on ACT hides under PE work; the softmax epilogue of superblock
sb-1 (GpSimd partition all-reduce -> DVE reciprocal -> DVE muls -> DMA) uses
no PE instructions at all and is interleaved into superblock sb, so the PE
runs matmuls back-to-back at the fp16 roofline for the whole kernel.
Input DMAs are HWDGE, chunked and ordered by first use.
"""

import os
import sys

import numpy as np

for _p in ("/opt/trn_rl_repo", "/root/.axon_site/_ro/trn_rl_repo"):
    if os.path.isdir(_p) and _p not in sys.path:
        sys.path.insert(0, _p)

import concourse.bass as bass
from concourse import bacc
import concourse.tile as tile
from concourse import mybir
from concourse.bass_utils import run_bass_kernel_spmd

B = 8
D = 256
S = 4096
HALF = 128            # partition-dim tile of D
N_DH = D // HALF      # 2 halves of the head dim
QSB = 512             # query superblock (free dim of every matmul)
NSB = S // QSB        # 8 superblocks
NJ = S // HALF        # 32 key chunks
SCALE = float(D) ** -0.5

FP32 = mybir.dt.float32
FP16 = mybir.dt.float16

# Set by test harness to capture a profile; harness-default is plain run.
TRACE = False
LAST_RESULT = None


def _build_attention():
    """One-core program; identical on all 8 cores (pure data parallel)."""
    nc = bacc.Bacc("TRN2")
    q16 = nc.dram_tensor("q16", [D, S], FP16, kind="ExternalInput")
    k16 = nc.dram_tensor("k16", [D, S], FP16, kind="ExternalInput")
    # v16[p, j*256 + d] = v[j*128 + p, d]  (host pre-tiled)
    v16 = nc.dram_tensor("v16", [HALF, NJ * D], FP16, kind="ExternalInput")
    out = nc.dram_tensor("out", [D, S], FP32, kind="ExternalOutput")

    with tile.TileContext(nc) as tc:
        with (
            tc.tile_pool(name="inp", bufs=1) as inp,
            tc.tile_pool(name="expp", bufs=8) as expp,
            tc.tile_pool(name="daccp", bufs=2) as daccp,
            tc.tile_pool(name="onorm", bufs=2) as onorm,
            tc.tile_pool(name="ps_s", bufs=3, space="PSUM") as ps_s,
            tc.tile_pool(name="ps_av", bufs=2, space="PSUM") as ps_av,
            tc.tile_pool(name="ps_d", bufs=1, space="PSUM") as ps_d,
        ):
            # ---- input DMAs (HWDGE, sync + scalar rings, first-use order)
            kh = [inp.tile([HALF, S], FP16, tag=f"kh{dh}", name=f"kh{dh}")
                  for dh in range(N_DH)]
            qh = [inp.tile([HALF, S], FP16, tag=f"qh{dh}", name=f"qh{dh}")
                  for dh in range(N_DH)]
            vall = inp.tile([HALF, NJ * D], FP16, tag="vall", name="vall")

            def load_k(dh, c0, c1):
                nc.sync.dma_start(out=kh[dh][:, c0:c1],
                                  in_=k16[dh * HALF:(dh + 1) * HALF, c0:c1])

            def load_q(dh, c0, c1):
                nc.sync.dma_start(out=qh[dh][:, c0:c1],
                                  in_=q16[dh * HALF:(dh + 1) * HALF, c0:c1])

            def load_v(c0, c1):
                nc.scalar.dma_start(out=vall[:, c0:c1], in_=v16[:, c0:c1])

            # First-use order, split across the two HWDGE rings so the
            # first matmul's four inputs (k0/k1 cols 0:128, q0/q1 cols
            # 0:512) land in parallel.  dh=0 + v on the sync ring, dh=1 on
            # the scalar ring.
            def load_k1(c0, c1):
                nc.scalar.dma_start(out=kh[1][:, c0:c1],
                                    in_=k16[HALF:2 * HALF, c0:c1])

            def load_q1(c0, c1):
                nc.scalar.dma_start(out=qh[1][:, c0:c1],
                                    in_=q16[HALF:2 * HALF, c0:c1])

            load_k(0, 0, 128)       # sync ring
            load_k1(0, 128)         # scalar ring
            load_q(0, 0, 512)
            load_q1(0, 512)
            load_k(0, 128, 1024)
            load_k1(128, 1024)
            # ACT exp-table prewarm: after the leading scalar-ring DMA
            # triggers (the ~2.7us table load would delay them), before the
            # rest, so it still overlaps the DMA wait.
            warm_in = inp.tile([1, 8], FP32, tag="warm_in", name="warm_in")
            nc.vector.memset(warm_in, 0.0)
            warm_out = inp.tile([1, 8], FP32, tag="warm_out", name="warm_out")
            nc.scalar.activation(
                warm_out, warm_in, mybir.ActivationFunctionType.Exp
            )
            nc.sync.dma_start(out=vall[:, 0:512], in_=v16[:, 0:512])
            load_k(0, 1024, 2560)
            load_k1(1024, 2560)
            nc.sync.dma_start(out=vall[:, 512:1536], in_=v16[:, 512:1536])
            load_k(0, 2560, 4096)
            load_k1(2560, 4096)
            nc.sync.dma_start(out=vall[:, 1536:3072], in_=v16[:, 1536:3072])
            nc.sync.dma_start(out=vall[:, 3072:5120], in_=v16[:, 3072:5120])
            nc.sync.dma_start(out=vall[:, 5120:8192], in_=v16[:, 5120:8192])
            load_q(0, 512, 1024)
            load_q1(512, 1024)
            load_q(0, 1024, 2560)
            load_q1(1024, 2560)
            load_q(0, 2560, 4096)
            load_q1(2560, 4096)

            # ---- PE HAM prewarm: dummy matmuls during the DMA wait -------
            warm_rhs = inp.tile([HALF, QSB], FP16, tag="warm_rhs",
                                name="warm_rhs")
            nc.vector.memset(warm_rhs, 0.0)
            warm_ps = ps_d.tile([HALF, QSB], FP32, tag="dn", name="warm_ps")
            for _ in range(8):
                nc.tensor.matmul(warm_ps, warm_rhs[:, 0:HALF], warm_rhs,
                                 start=True, stop=True)

            # constants for the PE-based final-superblock epilogue
            ones_h = inp.tile([HALF, 1], FP16, tag="ones_h", name="ones_h")
            nc.vector.memset(ones_h, 1.0)
            ones_b = inp.tile([1, HALF], FP32, tag="ones_b", name="ones_b")
            nc.vector.memset(ones_b, 1.0)

            # ---- main attention loop ------------------------------------
            def emit_scores(sb, j, sps):
                qs = slice(sb * QSB, (sb + 1) * QSB)
                js = slice(j * HALF, (j + 1) * HALF)
                sp = ps_s.tile([HALF, QSB], FP32, tag="sp", name="sp")
                for dh in range(N_DH):
                    nc.tensor.matmul(
                        sp, kh[dh][:, js], qh[dh][:, qs],
                        start=(dh == 0), stop=(dh == N_DH - 1),
                    )
                sps[j] = sp

            prev = None  # epilogue state of previous superblock
            for sb in range(NSB):
                qs = slice(sb * QSB, (sb + 1) * QSB)
                av = [ps_av.tile([HALF, QSB], FP32, tag=f"av{dh}",
                                 name=f"av{dh}") for dh in range(N_DH)]
                dacc = daccp.tile([HALF, QSB], FP16, tag="dacc", name="dacc")
                sps = {}
                emit_scores(sb, 0, sps)
                emit_scores(sb, 1, sps)
                for j in range(NJ):
                    if j + 2 < NJ:
                        emit_scores(sb, j + 2, sps)
                    et = expp.tile([HALF, QSB], FP16, tag="et", name="et")
                    nc.scalar.activation(
                        et, sps.pop(j), mybir.ActivationFunctionType.Exp,
                        scale=SCALE,
                    )
                    if j == 0:
                        nc.vector.tensor_copy(dacc, et)
                    else:
                        nc.vector.tensor_add(dacc, dacc, et)
                    for dh in range(N_DH):
                        nc.tensor.matmul(
                            av[dh],
                            vall[:, j * D + dh * HALF:j * D + (dh + 1) * HALF],
                            et,
                            start=(j == 0),
                            stop=(j == NJ - 1),
                        )
                    # previous superblock's softmax epilogue: PE-free chain
                    # (GpSimd reduce -> DVE recip -> DVE mul -> DMA), spread
                    # over early j so DVE never queues behind a long wait
                    if prev is not None:
                        if j == 1:
                            _epi_reduce(nc, onorm, prev)
                        elif j == 4:
                            _epi_recip(nc, onorm, prev)
                        elif j == 6:
                            _epi_norm(nc, onorm, out, prev)
                            prev = None
                prev = {"av": av, "dacc": dacc, "qs": qs}

            # Final superblock: the PE is idle now, so the matmul-based
            # reduce/broadcast chain (dred -> recip -> rb) is ~3us faster
            # than the GpSimd all-reduce; output in 256-col chunks so the
            # DMA write receipts overlap the remaining muls.
            dred = ps_d.tile([1, QSB], FP32, tag="dn", name="dred")
            nc.tensor.matmul(dred, ones_h, prev["dacc"], start=True,
                             stop=True)
            dr = onorm.tile([1, QSB], FP32, tag="dr", name="dr")
            nc.vector.reciprocal_approx_fast(dr, dred)
            rb = ps_d.tile([HALF, QSB], FP32, tag="dn", name="rb")
            nc.tensor.matmul(rb, ones_b, dr, start=True, stop=True)
            rbs = onorm.tile([HALF, QSB], FP32, tag="rbs", name="rbs")
            nc.scalar.copy(rbs, rb)  # ACT is idle here; frees DVE for muls
            for c in range(2):
                cs = slice(c * (QSB // 2), (c + 1) * (QSB // 2))
                oqs = slice(prev["qs"].start + c * (QSB // 2),
                            prev["qs"].start + (c + 1) * (QSB // 2))
                for dh in range(N_DH):
                    ot = onorm.tile([HALF, QSB // 2], FP32, tag="ot",
                                    name="ot", bufs=4)
                    nc.vector.tensor_mul(ot, prev["av"][dh][:, cs],
                                         rbs[:, cs])
                    nc.sync.dma_start(
                        out=out[dh * HALF:(dh + 1) * HALF, oqs], in_=ot
                    )
    nc.finalize()
    return nc


def _epi_reduce(nc, onorm, st):
    """denominator: all-reduce dacc across partitions on GpSimd."""
    denb = onorm.tile([HALF, QSB], FP32, tag="denb", name="denb")
    nc.gpsimd.partition_all_reduce(
        denb, st["dacc"], HALF, bass.bass_isa.ReduceOp.add
    )
    st["denb"] = denb


def _epi_recip(nc, onorm, st):
    """1/denom on DVE (fast Newton-Raphson approx, ~51 ULP)."""
    rinv = onorm.tile([HALF, QSB], FP32, tag="rinv", name="rinv")
    nc.vector.reciprocal_approx_fast(rinv, st["denb"])
    st["rinv"] = rinv


def _epi_norm(nc, onorm, out, st):
    """normalize AV by 1/denom and DMA the output block."""
    for dh in range(N_DH):
        ot = onorm.tile([HALF, QSB], FP32, tag="ot", name="ot", bufs=4)
        nc.vector.tensor_mul(ot, st["av"][dh], st["rinv"])
        nc.sync.dma_start(
            out=out[dh * HALF:(dh + 1) * HALF, st["qs"]], in_=ot
        )


_NC_CACHE = {}


def _get_program():
    if "nc" not in _NC_CACHE:
        _NC_CACHE["nc"] = _build_attention()
    return _NC_CACHE["nc"]


def kernel(queries, keys, values, q_pos, k_pos):
    global LAST_RESULT
    q = np.asarray(queries, dtype=np.float32).reshape(B, D, S)
    k = np.asarray(keys, dtype=np.float32).reshape(B, D, S)
    v = np.asarray(values, dtype=np.float32).reshape(B, D, S)
    qpt = np.asarray(q_pos, np.float32).reshape(S, D).T      # (D, S)
    kpt = np.asarray(k_pos, np.float32).reshape(S, D).T
    q16 = (q + qpt).astype(np.float16)                       # (B, D, S)
    k16 = (k + kpt).astype(np.float16)
    # v16[b][p, j*256+d] = v[b].T[j*128+p, d]
    v16 = np.ascontiguousarray(
        v.transpose(0, 2, 1).reshape(B, NJ, HALF, D).transpose(0, 2, 1, 3)
    ).reshape(B, HALF, NJ * D).astype(np.float16)

    nc = _get_program()
    in_maps = [
        {
            "q16": np.ascontiguousarray(q16[b]),
            "k16": np.ascontiguousarray(k16[b]),
            "v16": np.ascontiguousarray(v16[b]),
        }
        for b in range(B)
    ]
    res = run_bass_kernel_spmd(nc, in_maps, list(range(B)), trace=TRACE)
    LAST_RESULT = res
    out = np.stack([res.results[b]["out"] for b in range(B)])  # (B, D, S)
    return out.reshape(B, D, 64, 64).astype(np.float32)


# revision 11
# speedup vs baseline: 1.0109x; 1.0109x over previous
"""Trainium2 Bass kernel for KerasCrossAttentionModule (B=8, S=4096, D=256).

Sharding: data-parallel over batch B across 8 NeuronCores (1 batch/core).

Host prep (layout only, same class as the host transposes the problem needs
anyway): pos embeddings pre-added, operands cast to fp16, V pre-tiled so the
device sees three dense fp16 streams.

Per-core device math:
    scoresT[k, i] = sum_d kh[d, k] * qh[d, i]       (PE, fp16 in / fp32 acc)
    E = exp(scale * scoresT)                        (ACT, fp32 -> fp16)
    denom[i] = sum_k E[k, i]                        (DVE adds + GpSimd reduce)
    outT[d, i] = sum_k v[k, d] * E[k, i] / denom[i] (PE + DVE)

Pipelining: scores are emitted 2 key-chunks ahead of the AV matmuls so the
exp() latency on ACT hides under PE work; the softmax epilogue of superblock
sb-1 (GpSimd partition all-reduce -> DVE reciprocal -> DVE muls -> DMA) uses
no PE instructions at all and is interleaved into superblock sb, so the PE
runs matmuls back-to-back at the fp16 roofline for the whole kernel.
Input DMAs are HWDGE, chunked and ordered by first use.
"""

import os
import sys

import numpy as np

for _p in ("/opt/trn_rl_repo", "/root/.axon_site/_ro/trn_rl_repo"):
    if os.path.isdir(_p) and _p not in sys.path:
        sys.path.insert(0, _p)

import concourse.bass as bass
from concourse import bacc
import concourse.tile as tile
from concourse import mybir
from concourse.bass_utils import run_bass_kernel_spmd

B = 8
D = 256
S = 4096
HALF = 128            # partition-dim tile of D
N_DH = D // HALF      # 2 halves of the head dim
QSB = 512             # query superblock (free dim of every matmul)
NSB = S // QSB        # 8 superblocks
NJ = S // HALF        # 32 key chunks
SCALE = float(D) ** -0.5

FP32 = mybir.dt.float32
FP16 = mybir.dt.float16

# Set by test harness to capture a profile; harness-default is plain run.
TRACE = False
LAST_RESULT = None


def _build_attention():
    """One-core program; identical on all 8 cores (pure data parallel)."""
    nc = bacc.Bacc("TRN2")
    q16 = nc.dram_tensor("q16", [D, S], FP16, kind="ExternalInput")
    k16 = nc.dram_tensor("k16", [D, S], FP16, kind="ExternalInput")
    # v16[p, j*256 + d] = v[j*128 + p, d]  (host pre-tiled)
    v16 = nc.dram_tensor("v16", [HALF, NJ * D], FP16, kind="ExternalInput")
    out = nc.dram_tensor("out", [D, S], FP32, kind="ExternalOutput")

    with tile.TileContext(nc) as tc:
        with (
            tc.tile_pool(name="inp", bufs=1) as inp,
            tc.tile_pool(name="expp", bufs=8) as expp,
            tc.tile_pool(name="daccp", bufs=2) as daccp,
            tc.tile_pool(name="onorm", bufs=2) as onorm,
            tc.tile_pool(name="ps_s", bufs=3, space="PSUM") as ps_s,
            tc.tile_pool(name="ps_av", bufs=2, space="PSUM") as ps_av,
            tc.tile_pool(name="ps_d", bufs=1, space="PSUM") as ps_d,
        ):
            # ---- input DMAs (HWDGE, sync + scalar rings, first-use order)
            kh = [inp.tile([HALF, S], FP16, tag=f"kh{dh}", name=f"kh{dh}")
                  for dh in range(N_DH)]
            qh = [inp.tile([HALF, S], FP16, tag=f"qh{dh}", name=f"qh{dh}")
                  for dh in range(N_DH)]
            vall = inp.tile([HALF, NJ * D], FP16, tag="vall", name="vall")

            def load_k(dh, c0, c1):
                nc.sync.dma_start(out=kh[dh][:, c0:c1],
                                  in_=k16[dh * HALF:(dh + 1) * HALF, c0:c1])

            def load_q(dh, c0, c1):
                nc.sync.dma_start(out=qh[dh][:, c0:c1],
                                  in_=q16[dh * HALF:(dh + 1) * HALF, c0:c1])

            def load_v(c0, c1):
                nc.scalar.dma_start(out=vall[:, c0:c1], in_=v16[:, c0:c1])

            # First-use order (v4 layout, empirically best): all k+q on
            # the sync ring, v on the scalar ring.  The ACT exp-table
            # prewarm is first on the ACT queue so the ~2.7us table load
            # overlaps the input DMA wait (v triggers follow it).
            warm_in = inp.tile([1, 8], FP32, tag="warm_in", name="warm_in")
            nc.vector.memset(warm_in, 0.0)
            warm_out = inp.tile([1, 8], FP32, tag="warm_out", name="warm_out")
            nc.scalar.activation(
                warm_out, warm_in, mybir.ActivationFunctionType.Exp
            )
            load_k(0, 0, 128)
            load_k(1, 0, 128)
            load_q(0, 0, 512)
            load_q(1, 0, 512)
            load_k(0, 128, 1024)
            load_k(1, 128, 1024)
            load_v(0, 512)
            load_k(0, 1024, 2560)
            load_k(1, 1024, 2560)
            load_v(512, 1536)
            load_k(0, 2560, 4096)
            load_k(1, 2560, 4096)
            load_v(1536, 3072)
            load_v(3072, 5120)
            load_v(5120, 8192)
            load_q(0, 512, 1024)
            load_q(1, 512, 1024)
            load_q(0, 1024, 2560)
            load_q(1, 1024, 2560)
            load_q(0, 2560, 4096)
            load_q(1, 2560, 4096)

            # ---- PE HAM prewarm: dummy matmuls during the DMA wait -------
            warm_rhs = inp.tile([HALF, QSB], FP16, tag="warm_rhs",
                                name="warm_rhs")
            nc.vector.memset(warm_rhs, 0.0)
            warm_ps = ps_d.tile([HALF, QSB], FP32, tag="dn", name="warm_ps")
            for _ in range(8):
                nc.tensor.matmul(warm_ps, warm_rhs[:, 0:HALF], warm_rhs,
                                 start=True, stop=True)

            # constants for the PE-based final-superblock epilogue
            ones_h = inp.tile([HALF, 1], FP16, tag="ones_h", name="ones_h")
            nc.vector.memset(ones_h, 1.0)
            ones_b = inp.tile([1, HALF], FP32, tag="ones_b", name="ones_b")
            nc.vector.memset(ones_b, 1.0)

            # ---- main attention loop ------------------------------------
            def emit_scores(sb, j, sps):
                qs = slice(sb * QSB, (sb + 1) * QSB)
                js = slice(j * HALF, (j + 1) * HALF)
                sp = ps_s.tile([HALF, QSB], FP32, tag="sp", name="sp")
                for dh in range(N_DH):
                    nc.tensor.matmul(
                        sp, kh[dh][:, js], qh[dh][:, qs],
                        start=(dh == 0), stop=(dh == N_DH - 1),
                    )
                sps[j] = sp

            prev = None  # epilogue state of previous superblock
            for sb in range(NSB):
                qs = slice(sb * QSB, (sb + 1) * QSB)
                av = [ps_av.tile([HALF, QSB], FP32, tag=f"av{dh}",
                                 name=f"av{dh}") for dh in range(N_DH)]
                dacc = daccp.tile([HALF, QSB], FP16, tag="dacc", name="dacc")
                sps = {}
                emit_scores(sb, 0, sps)
                emit_scores(sb, 1, sps)
                for j in range(NJ):
                    if j + 2 < NJ:
                        emit_scores(sb, j + 2, sps)
                    et = expp.tile([HALF, QSB], FP16, tag="et", name="et")
                    nc.scalar.activation(
                        et, sps.pop(j), mybir.ActivationFunctionType.Exp,
                        scale=SCALE,
                    )
                    if j == 0:
                        nc.vector.tensor_copy(dacc, et)
                    else:
                        nc.vector.tensor_add(dacc, dacc, et)
                    for dh in range(N_DH):
                        nc.tensor.matmul(
                            av[dh],
                            vall[:, j * D + dh * HALF:j * D + (dh + 1) * HALF],
                            et,
                            start=(j == 0),
                            stop=(j == NJ - 1),
                        )
                    # previous superblock's softmax epilogue: PE-free chain
                    # (GpSimd reduce -> DVE recip -> DVE mul -> DMA), spread
                    # over early j so DVE never queues behind a long wait
                    if prev is not None:
                        if j == 1:
                            _epi_reduce(nc, onorm, prev)
                        elif j == 4:
                            _epi_recip(nc, onorm, prev)
                        elif j == 6:
                            _epi_norm(nc, onorm, out, prev)
                            prev = None
                prev = {"av": av, "dacc": dacc, "qs": qs}

            # Final superblock: the PE is idle now, so the matmul-based
            # reduce/broadcast chain (dred -> recip -> rb) is ~3us faster
            # than the GpSimd all-reduce; output in 256-col chunks so the
            # DMA write receipts overlap the remaining muls.
            dred = ps_d.tile([1, QSB], FP32, tag="dn", name="dred")
            nc.tensor.matmul(dred, ones_h, prev["dacc"], start=True,
                             stop=True)
            dr = onorm.tile([1, QSB], FP32, tag="dr", name="dr")
            nc.vector.reciprocal_approx_fast(dr, dred)
            rb = ps_d.tile([HALF, QSB], FP32, tag="dn", name="rb")
            nc.tensor.matmul(rb, ones_b, dr, start=True, stop=True)
            rbs = onorm.tile([HALF, QSB], FP32, tag="rbs", name="rbs")
            nc.scalar.copy(rbs, rb)  # ACT is idle here; frees DVE for muls
            for c in range(2):
                cs = slice(c * (QSB // 2), (c + 1) * (QSB // 2))
                oqs = slice(prev["qs"].start + c * (QSB // 2),
                            prev["qs"].start + (c + 1) * (QSB // 2))
                for dh in range(N_DH):
                    ot = onorm.tile([HALF, QSB // 2], FP32, tag="ot",
                                    name="ot", bufs=4)
                    nc.vector.tensor_mul(ot, prev["av"][dh][:, cs],
                                         rbs[:, cs])
                    nc.sync.dma_start(
                        out=out[dh * HALF:(dh + 1) * HALF, oqs], in_=ot
                    )
    nc.finalize()
    return nc


def _epi_reduce(nc, onorm, st):
    """denominator: all-reduce dacc across partitions on GpSimd."""
    denb = onorm.tile([HALF, QSB], FP32, tag="denb", name="denb")
    nc.gpsimd.partition_all_reduce(
        denb, st["dacc"], HALF, bass.bass_isa.ReduceOp.add
    )
    st["denb"] = denb


def _epi_recip(nc, onorm, st):
    """1/denom on DVE (fast Newton-Raphson approx, ~51 ULP)."""
    rinv = onorm.tile([HALF, QSB], FP32, tag="rinv", name="rinv")
    nc.vector.reciprocal_approx_fast(rinv, st["denb"])
    st["rinv"] = rinv


def _epi_norm(nc, onorm, out, st):
    """normalize AV by 1/denom and DMA the output block."""
    for dh in range(N_DH):
        ot = onorm.tile([HALF, QSB], FP32, tag="ot", name="ot", bufs=4)
        nc.vector.tensor_mul(ot, st["av"][dh], st["rinv"])
        nc.sync.dma_start(
            out=out[dh * HALF:(dh + 1) * HALF, st["qs"]], in_=ot
        )


_NC_CACHE = {}


def _get_program():
    if "nc" not in _NC_CACHE:
        _NC_CACHE["nc"] = _build_attention()
    return _NC_CACHE["nc"]


def kernel(queries, keys, values, q_pos, k_pos):
    global LAST_RESULT
    q = np.asarray(queries, dtype=np.float32).reshape(B, D, S)
    k = np.asarray(keys, dtype=np.float32).reshape(B, D, S)
    v = np.asarray(values, dtype=np.float32).reshape(B, D, S)
    qpt = np.asarray(q_pos, np.float32).reshape(S, D).T      # (D, S)
    kpt = np.asarray(k_pos, np.float32).reshape(S, D).T
    q16 = (q + qpt).astype(np.float16)                       # (B, D, S)
    k16 = (k + kpt).astype(np.float16)
    # v16[b][p, j*256+d] = v[b].T[j*128+p, d]
    v16 = np.ascontiguousarray(
        v.transpose(0, 2, 1).reshape(B, NJ, HALF, D).transpose(0, 2, 1, 3)
    ).reshape(B, HALF, NJ * D).astype(np.float16)

    nc = _get_program()
    in_maps = [
        {
            "q16": np.ascontiguousarray(q16[b]),
            "k16": np.ascontiguousarray(k16[b]),
            "v16": np.ascontiguousarray(v16[b]),
        }
        for b in range(B)
    ]
    res = run_bass_kernel_spmd(nc, in_maps, list(range(B)), trace=TRACE)
    LAST_RESULT = res
    out = np.stack([res.results[b]["out"] for b in range(B)])  # (B, D, S)
    return out.reshape(B, D, 64, 64).astype(np.float32)


# revision 12
# speedup vs baseline: 1.0169x; 1.0059x over previous
"""Trainium2 Bass kernel for KerasCrossAttentionModule (B=8, S=4096, D=256).

Sharding: data-parallel over batch B across 8 NeuronCores (1 batch/core).

Host prep (layout only, same class as the host transposes the problem needs
anyway): pos embeddings pre-added, operands cast to fp16, V pre-tiled so the
device sees three dense fp16 streams.

Per-core device math:
    scoresT[k, i] = sum_d kh[d, k] * qh[d, i]       (PE, fp16 in / fp32 acc)
    E = exp(scale * scoresT)                        (ACT, fp32 -> fp16)
    denom[i] = sum_k E[k, i]                        (DVE adds + GpSimd reduce)
    outT[d, i] = sum_k v[k, d] * E[k, i] / denom[i] (PE + DVE)

Pipelining: scores are emitted 2 key-chunks ahead of the AV matmuls so the
exp() latency on ACT hides under PE work; the softmax epilogue of superblock
sb-1 (GpSimd partition all-reduce -> DVE reciprocal -> DVE muls -> DMA) uses
no PE instructions at all and is interleaved into superblock sb, so the PE
runs matmuls back-to-back at the fp16 roofline for the whole kernel.
Input DMAs are HWDGE, chunked and ordered by first use.
"""

import os
import sys

import numpy as np

for _p in ("/opt/trn_rl_repo", "/root/.axon_site/_ro/trn_rl_repo"):
    if os.path.isdir(_p) and _p not in sys.path:
        sys.path.insert(0, _p)

import concourse.bass as bass
from concourse import bacc
import concourse.tile as tile
from concourse import mybir
from concourse.bass_utils import run_bass_kernel_spmd

B = 8
D = 256
S = 4096
HALF = 128            # partition-dim tile of D
N_DH = D // HALF      # 2 halves of the head dim
QSB = 512             # query superblock (free dim of every matmul)
NSB = S // QSB        # 8 superblocks
NJ = S // HALF        # 32 key chunks
SCALE = float(D) ** -0.5

FP32 = mybir.dt.float32
FP16 = mybir.dt.float16

# Set by test harness to capture a profile; harness-default is plain run.
TRACE = False
LAST_RESULT = None


def _build_attention():
    """One-core program; identical on all 8 cores (pure data parallel)."""
    nc = bacc.Bacc("TRN2")
    q16 = nc.dram_tensor("q16", [D, S], FP16, kind="ExternalInput")
    k16 = nc.dram_tensor("k16", [D, S], FP16, kind="ExternalInput")
    # v16[p, j*256 + d] = v[j*128 + p, d]  (host pre-tiled)
    v16 = nc.dram_tensor("v16", [HALF, NJ * D], FP16, kind="ExternalInput")
    out = nc.dram_tensor("out", [D, S], FP32, kind="ExternalOutput")

    with tile.TileContext(nc) as tc:
        with (
            tc.tile_pool(name="inp", bufs=1) as inp,
            tc.tile_pool(name="expp", bufs=8) as expp,
            tc.tile_pool(name="daccp", bufs=2) as daccp,
            tc.tile_pool(name="onorm", bufs=2) as onorm,
            tc.tile_pool(name="ps_s", bufs=3, space="PSUM") as ps_s,
            tc.tile_pool(name="ps_av", bufs=2, space="PSUM") as ps_av,
            tc.tile_pool(name="ps_d", bufs=1, space="PSUM") as ps_d,
        ):
            # ---- input DMAs (HWDGE, sync + scalar rings, first-use order)
            kh = [inp.tile([HALF, S], FP16, tag=f"kh{dh}", name=f"kh{dh}")
                  for dh in range(N_DH)]
            qh = [inp.tile([HALF, S], FP16, tag=f"qh{dh}", name=f"qh{dh}")
                  for dh in range(N_DH)]
            vall = inp.tile([HALF, NJ * D], FP16, tag="vall", name="vall")

            def load_k(dh, c0, c1):
                nc.sync.dma_start(out=kh[dh][:, c0:c1],
                                  in_=k16[dh * HALF:(dh + 1) * HALF, c0:c1])

            def load_q(dh, c0, c1):
                nc.sync.dma_start(out=qh[dh][:, c0:c1],
                                  in_=q16[dh * HALF:(dh + 1) * HALF, c0:c1])

            def load_v(c0, c1):
                nc.scalar.dma_start(out=vall[:, c0:c1], in_=v16[:, c0:c1])

            # First-use order (v4 layout, empirically best): all k+q on
            # the sync ring, v on the scalar ring.  The ACT exp-table
            # prewarm is first on the ACT queue so the ~2.7us table load
            # overlaps the input DMA wait (v triggers follow it).
            warm_in = inp.tile([1, 8], FP32, tag="warm_in", name="warm_in")
            nc.vector.memset(warm_in, 0.0)
            warm_out = inp.tile([1, 8], FP32, tag="warm_out", name="warm_out")
            nc.scalar.activation(
                warm_out, warm_in, mybir.ActivationFunctionType.Exp
            )
            load_k(0, 0, 128)
            load_k(1, 0, 128)
            load_q(0, 0, 512)
            load_q(1, 0, 512)
            load_k(0, 128, 512)
            load_k(1, 128, 512)
            load_k(0, 512, 1024)
            load_k(1, 512, 1024)
            load_v(0, 512)
            load_k(0, 1024, 2560)
            load_k(1, 1024, 2560)
            load_v(512, 1536)
            load_k(0, 2560, 4096)
            load_k(1, 2560, 4096)
            load_v(1536, 3072)
            load_v(3072, 5120)
            load_v(5120, 8192)
            load_q(0, 512, 1024)
            load_q(1, 512, 1024)
            load_q(0, 1024, 2560)
            load_q(1, 1024, 2560)
            load_q(0, 2560, 4096)
            load_q(1, 2560, 4096)

            # ---- PE HAM prewarm: dummy matmuls during the DMA wait -------
            warm_rhs = inp.tile([HALF, QSB], FP16, tag="warm_rhs",
                                name="warm_rhs")
            nc.vector.memset(warm_rhs, 0.0)
            warm_ps = ps_d.tile([HALF, QSB], FP32, tag="dn", name="warm_ps")
            for _ in range(12):
                nc.tensor.matmul(warm_ps, warm_rhs[:, 0:HALF], warm_rhs,
                                 start=True, stop=True)

            # constants for the PE-based final-superblock epilogue
            ones_h = inp.tile([HALF, 1], FP16, tag="ones_h", name="ones_h")
            nc.vector.memset(ones_h, 1.0)
            ones_b = inp.tile([1, HALF], FP32, tag="ones_b", name="ones_b")
            nc.vector.memset(ones_b, 1.0)

            # ---- main attention loop ------------------------------------
            def emit_scores(sb, j, sps):
                qs = slice(sb * QSB, (sb + 1) * QSB)
                js = slice(j * HALF, (j + 1) * HALF)
                sp = ps_s.tile([HALF, QSB], FP32, tag="sp", name="sp")
                for dh in range(N_DH):
                    nc.tensor.matmul(
                        sp, kh[dh][:, js], qh[dh][:, qs],
                        start=(dh == 0), stop=(dh == N_DH - 1),
                    )
                sps[j] = sp

            prev = None  # epilogue state of previous superblock
            for sb in range(NSB):
                qs = slice(sb * QSB, (sb + 1) * QSB)
                av = [ps_av.tile([HALF, QSB], FP32, tag=f"av{dh}",
                                 name=f"av{dh}") for dh in range(N_DH)]
                dacc = daccp.tile([HALF, QSB], FP16, tag="dacc", name="dacc")
                sps = {}
                emit_scores(sb, 0, sps)
                emit_scores(sb, 1, sps)
                for j in range(NJ):
                    if j + 2 < NJ:
                        emit_scores(sb, j + 2, sps)
                    et = expp.tile([HALF, QSB], FP16, tag="et", name="et")
                    nc.scalar.activation(
                        et, sps.pop(j), mybir.ActivationFunctionType.Exp,
                        scale=SCALE,
                    )
                    if j == 0:
                        nc.vector.tensor_copy(dacc, et)
                    else:
                        nc.vector.tensor_add(dacc, dacc, et)
                    for dh in range(N_DH):
                        nc.tensor.matmul(
                            av[dh],
                            vall[:, j * D + dh * HALF:j * D + (dh + 1) * HALF],
                            et,
                            start=(j == 0),
                            stop=(j == NJ - 1),
                        )
                    # previous superblock's softmax epilogue: PE-free chain
                    # (GpSimd reduce -> DVE recip -> DVE mul -> DMA), spread
                    # over early j so DVE never queues behind a long wait
                    if prev is not None:
                        if j == 1:
                            _epi_reduce(nc, onorm, prev)
                        elif j == 4:
                            _epi_recip(nc, onorm, prev)
                        elif j == 6:
                            _epi_norm(nc, onorm, out, prev)
                            prev = None
                prev = {"av": av, "dacc": dacc, "qs": qs}

            # Final superblock: the PE is idle now, so the matmul-based
            # reduce/broadcast chain (dred -> recip -> rb) is ~3us faster
            # than the GpSimd all-reduce; output in 256-col chunks so the
            # DMA write receipts overlap the remaining muls.
            dred = ps_d.tile([1, QSB], FP32, tag="dn", name="dred")
            nc.tensor.matmul(dred, ones_h, prev["dacc"], start=True,
                             stop=True)
            dr = onorm.tile([1, QSB], FP32, tag="dr", name="dr")
            nc.vector.reciprocal_approx_fast(dr, dred)
            # av -> SBUF copies overlap the dred/recip/rb chain, so the
            # muls can read rb straight from PSUM (one PSUM operand only)
            avs = []
            for dh in range(N_DH):
                a = onorm.tile([HALF, QSB], FP32, tag="avs", name="avs")
                nc.vector.tensor_copy(a, prev["av"][dh])
                avs.append(a)
            rb = ps_d.tile([HALF, QSB], FP32, tag="dn", name="rb")
            nc.tensor.matmul(rb, ones_b, dr, start=True, stop=True)
            for c in range(2):
                cs = slice(c * (QSB // 2), (c + 1) * (QSB // 2))
                oqs = slice(prev["qs"].start + c * (QSB // 2),
                            prev["qs"].start + (c + 1) * (QSB // 2))
                for dh in range(N_DH):
                    ot = onorm.tile([HALF, QSB // 2], FP32, tag="ot",
                                    name="ot", bufs=4)
                    nc.vector.tensor_mul(ot, avs[dh][:, cs], rb[:, cs])
                    nc.sync.dma_start(
                        out=out[dh * HALF:(dh + 1) * HALF, oqs], in_=ot
                    )
    nc.finalize()
    return nc


def _epi_reduce(nc, onorm, st):
    """denominator: all-reduce dacc across partitions on GpSimd."""
    denb = onorm.tile([HALF, QSB], FP32, tag="denb", name="denb")
    nc.gpsimd.partition_all_reduce(
        denb, st["dacc"], HALF, bass.bass_isa.ReduceOp.add
    )
    st["denb"] = denb


def _epi_recip(nc, onorm, st):
    """1/denom on DVE (fast Newton-Raphson approx, ~51 ULP)."""
    rinv = onorm.tile([HALF, QSB], FP32, tag="rinv", name="rinv")
    nc.vector.reciprocal_approx_fast(rinv, st["denb"])
    st["rinv"] = rinv


def _epi_norm(nc, onorm, out, st):
    """normalize AV by 1/denom and DMA the output block."""
    for dh in range(N_DH):
        ot = onorm.tile([HALF, QSB], FP32, tag="ot", name="ot", bufs=4)
        nc.vector.tensor_mul(ot, st["av"][dh], st["rinv"])
        nc.sync.dma_start(
            out=out[dh * HALF:(dh + 1) * HALF, st["qs"]], in_=ot
        )


_NC_CACHE = {}


def _get_program():
    if "nc" not in _NC_CACHE:
        _NC_CACHE["nc"] = _build_attention()
    return _NC_CACHE["nc"]


def kernel(queries, keys, values, q_pos, k_pos):
    global LAST_RESULT
    q = np.asarray(queries, dtype=np.float32).reshape(B, D, S)
    k = np.asarray(keys, dtype=np.float32).reshape(B, D, S)
    v = np.asarray(values, dtype=np.float32).reshape(B, D, S)
    qpt = np.asarray(q_pos, np.float32).reshape(S, D).T      # (D, S)
    kpt = np.asarray(k_pos, np.float32).reshape(S, D).T
    q16 = (q + qpt).astype(np.float16)                       # (B, D, S)
    k16 = (k + kpt).astype(np.float16)
    # v16[b][p, j*256+d] = v[b].T[j*128+p, d]
    v16 = np.ascontiguousarray(
        v.transpose(0, 2, 1).reshape(B, NJ, HALF, D).transpose(0, 2, 1, 3)
    ).reshape(B, HALF, NJ * D).astype(np.float16)

    nc = _get_program()
    in_maps = [
        {
            "q16": np.ascontiguousarray(q16[b]),
            "k16": np.ascontiguousarray(k16[b]),
            "v16": np.ascontiguousarray(v16[b]),
        }
        for b in range(B)
    ]
    res = run_bass_kernel_spmd(nc, in_maps, list(range(B)), trace=TRACE)
    LAST_RESULT = res
    out = np.stack([res.results[b]["out"] for b in range(B)])  # (B, D, S)
    return out.reshape(B, D, 64, 64).astype(np.float32)
